# revision 25
# baseline (speedup 1.0000x reference)
"""Sparse avg-pool (segment mean) for Trainium2, 8 NeuronCores — TensorEngine version.

Range-shard coarse ids across cores (core k owns ids [k*31360, (k+1)*31360)),
so no collective is needed.  On each core the segment-sum runs on the
TensorEngine: the host sorts the core's rows by local id and buckets them into
245 windows of 128 consecutive ids, padding each window to `cap` tiles of 128
tokens.  For each 128-token tile the DVE builds a one-hot [token, seg] matrix
(is_equal of the token's window-relative id against an iota row), and the PE
accumulates onehot^T @ [feats | 1] into a per-window [128, 66] PSUM bank in
f32 (bf16 multiplicands: features round once to bf16, counts are exact).  A
DVE epilogue divides sums by max(count, 1) and DMAs the window's 128 output
rows.  No GPSIMD scatter ucode anywhere — the old dma_scatter_add version was
descriptor-generation bound at ~7 ns/token.
"""
import os
import sys
from dataclasses import dataclass

sys.path.insert(0, "/opt/trn_rl_repo")

import numpy as np

NCORES = 8
C = 64
CW = 66  # 64 feats + count + window-relative id
W = 128  # ids per window


@dataclass(frozen=True)
class Cfg:
    n_coarse_pad: int = 250_880  # 8 * 245 * 128
    cap: int = 9                 # tiles of 128 tokens per window
    load_windows: int = 8        # windows per input DMA

    @property
    def rng(self):
        return self.n_coarse_pad // NCORES

    @property
    def n_win(self):  # windows per core
        return self.rng // W

    @property
    def s_slots(self):  # 128-token slots per core
        return self.n_win * self.cap

    @property
    def s_tot(self):
        return self.s_slots * 128


FUSED_OH = bool(int(os.environ.get("KERNEL_FUSED_OH", "1")))

CFG = Cfg()
_nc_cache = {}
LAST_RESULT = None


def build_nc(cfg: Cfg):
    from concourse import bacc, mybir, tile

    bf16 = mybir.dt.bfloat16
    f32 = mybir.dt.float32
    nc = bacc.Bacc("TRN2", target_bir_lowering=False)
    feats_ext = nc.declare_dram_parameter(
        "feats", [128, cfg.s_slots, CW], bf16, isOutput=False
    )
    iota_ext = nc.declare_dram_parameter("iota", [128, W], bf16, isOutput=False)
    out_ext = nc.declare_dram_parameter(
        "out", [cfg.n_win, W, C], f32, isOutput=True
    )

    lw = cfg.load_windows
    n_chunks = (cfg.n_win + lw - 1) // lw
    assert cfg.n_win % lw == 0 or True

    with tile.TileContext(nc) as tc:
        with (
            tc.tile_pool(name="stage", bufs=2) as stagep,
            tc.tile_pool(name="oh", bufs=4) as ohp,
            tc.tile_pool(name="psum", bufs=8, space="PSUM") as psump,
            tc.tile_pool(name="fin", bufs=4) as finp,
            tc.tile_pool(name="cst", bufs=1) as cstp,
        ):
            iota_t = cstp.tile([128, W], bf16)
            nc.sync.dma_start(out=iota_t[:], in_=iota_ext[:])

            for ch in range(n_chunks):
                w0 = ch * lw
                nw = min(lw, cfg.n_win - w0)
                src = stagep.tile([128, lw * cfg.cap, CW], bf16, tag="src")
                nc.sync.dma_start(
                    out=src[:, : nw * cfg.cap, :],
                    in_=feats_ext[:, w0 * cfg.cap : (w0 + nw) * cfg.cap, :],
                )
                for wi in range(nw):
                    w = w0 + wi
                    ps = psump.tile([128, CW], f32, tag="ps")
                    if FUSED_OH:
                        s0 = wi * cfg.cap
                        ohw = ohp.tile([128, cfg.cap, W], bf16, tag="ohw")
                        nc.vector.tensor_tensor(
                            out=ohw[:],
                            in0=src[:, s0 : s0 + cfg.cap, CW - 1 : CW].to_broadcast(
                                [128, cfg.cap, W]
                            ),
                            in1=iota_t[:].unsqueeze(1).to_broadcast(
                                [128, cfg.cap, W]
                            ),
                            op=mybir.AluOpType.is_equal,
                        )
                    for j in range(cfg.cap):
                        s = wi * cfg.cap + j
                        if FUSED_OH:
                            oh = ohw[:, j, :]
                        else:
                            oht = ohp.tile([128, W], bf16, tag="oh")
                            nc.vector.tensor_tensor(
                                out=oht[:],
                                in0=src[:, s, CW - 1 : CW].to_broadcast([128, W]),
                                in1=iota_t[:],
                                op=mybir.AluOpType.is_equal,
                            )
                            oh = oht[:]
                        nc.tensor.matmul(
                            out=ps[:],
                            lhsT=oh,
                            rhs=src[:, s, :CW],
                            start=(j == 0),
                            stop=(j == cfg.cap - 1),
                        )
                    den = finp.tile([128, 1], f32, tag="den")
                    nc.vector.tensor_scalar_max(den[:], ps[:, C : C + 1], 1.0)
                    inv = finp.tile([128, 1], f32, tag="inv")
                    nc.vector.reciprocal(inv[:], den[:])
                    ot = finp.tile([128, C], f32, tag="ot")
                    # multiply on the otherwise-idle ACT engine:
                    # out = Copy(in * scale), scale broadcast per partition
                    nc.scalar.activation(
                        ot[:], ps[:, :C], mybir.ActivationFunctionType.Copy,
                        scale=inv[:],
                    )
                    nc.sync.dma_start(out=out_ext[w], in_=ot[:])
    nc.compile()
    return nc


def shard_inputs(feats, ids, cfg: Cfg):
    """Host: route rows to owner cores, bucket into 128-id windows."""
    import ml_dtypes

    ids = np.asarray(ids, dtype=np.int64).ravel()
    feats = np.asarray(feats, dtype=np.float32)
    owner = ids // cfg.rng
    local = (ids - owner * cfg.rng).astype(np.int32)
    order = np.argsort(owner, kind="stable")
    counts = np.bincount(owner, minlength=NCORES)
    offs = np.zeros(NCORES + 1, np.int64)
    np.cumsum(counts, out=offs[1:])
    feats_sorted = feats[order]
    local_sorted = local[order]

    in_maps = []
    iota = np.broadcast_to(
        np.arange(W, dtype=np.float32), (128, W)
    ).astype(ml_dtypes.bfloat16)
    for k in range(NCORES):
        fk = feats_sorted[offs[k] : offs[k + 1]]
        lk = local_sorted[offs[k] : offs[k + 1]]
        n_k = lk.shape[0]
        fa = np.zeros((cfg.s_tot, CW), np.float32)
        if n_k:
            sorder = np.argsort(lk, kind="stable")
            ls = lk[sorder]
            win = ls >> 7
            wcount = np.bincount(win, minlength=cfg.n_win)
            assert wcount.max() <= cfg.cap * 128, (
                f"window overflow {wcount.max()} > {cfg.cap * 128}"
            )
            wstart = np.zeros(cfg.n_win, np.int64)
            np.cumsum(wcount[:-1], out=wstart[1:])
            rank_in_win = np.arange(n_k) - wstart[win]
            dst = win * (cfg.cap * 128) + rank_in_win
            fa[dst, :C] = fk[sorder]
            fa[dst, C] = 1.0
            fa[dst, C + 1] = (ls & 127).astype(np.float32)
        arranged = np.ascontiguousarray(
            fa.reshape(cfg.s_slots, 128, CW).transpose(1, 0, 2)
        ).astype(ml_dtypes.bfloat16)
        in_maps.append({"feats": arranged, "iota": iota})
    return in_maps


def assemble_output(results, n_coarse, cfg: Cfg):
    out = np.empty((NCORES * cfg.rng, C), np.float32)
    for k in range(NCORES):
        out[k * cfg.rng : (k + 1) * cfg.rng] = results[k]["out"].reshape(
            cfg.rng, C
        )
    return out[:n_coarse]


def emulate_device(in_map, cfg: Cfg):
    feats = np.asarray(in_map["feats"], dtype=np.float32)  # [128, s_slots, CW]
    acc = np.zeros((cfg.n_win, W, CW - 1), np.float64)
    for s in range(cfg.s_slots):
        w = s // cfg.cap
        for p in range(128):
            row = feats[p, s]
            seg = int(row[CW - 1])
            acc[w, seg, :] += row[: CW - 1]
    den = np.maximum(acc[:, :, C], 1.0)[:, :, None]
    return {"out": (acc[:, :, :C] / den).astype(np.float32)}


def _install_axon_hooks_shim():
    """Provide antenv.axon_hooks + the ctypes NTFF hook if the image lacks it.

    Mirrors trn_agent_boot.trn_boot._ntff_profile_via_ctypes so that
    run_bass_kernel_spmd(trace=True) can profile under axon.
    """
    import contextlib
    import ctypes
    import types

    try:
        from antenv.axon_hooks import get_axon_ntff_profile_hook  # noqa: F401

        return
    except ImportError:
        pass
    import antenv

    mod = types.ModuleType("antenv.axon_hooks")
    state = {"h": None}
    mod.set_axon_ntff_profile_hook = lambda h: state.__setitem__("h", h)
    mod.get_axon_ntff_profile_hook = lambda: state["h"]
    antenv.axon_hooks = mod
    sys.modules["antenv.axon_hooks"] = mod

    so_path = "/opt/axon/libaxon_pjrt.so"
    if not os.path.exists(so_path):
        return
    lib = ctypes.CDLL(so_path)
    if not hasattr(lib, "axon_start_nrt_profile"):
        return
    lib.axon_start_nrt_profile.argtypes = [
        ctypes.POINTER(ctypes.c_int64),
        ctypes.c_size_t,
    ]
    lib.axon_start_nrt_profile.restype = ctypes.c_int64
    lib.axon_stop_nrt_profile.argtypes = [ctypes.c_char_p]
    lib.axon_stop_nrt_profile.restype = ctypes.c_int64

    @contextlib.contextmanager
    def _hook(output_dir, device_ids):
        import jax

        jax.devices()
        if device_ids:
            ids = (ctypes.c_int64 * len(device_ids))(*device_ids)
            rc = lib.axon_start_nrt_profile(ids, len(device_ids))
        else:
            rc = lib.axon_start_nrt_profile(None, 0)
        if rc != 0:
            raise RuntimeError(f"axon_start_nrt_profile rc={rc}")
        try:
            yield
        finally:
            n = lib.axon_stop_nrt_profile(str(output_dir).encode())
            print(f"profile: {n} file(s) written to {output_dir}", file=sys.stderr)

    state["h"] = _hook


def kernel(fine_feats, coarse_ids, num_coarse):
    global LAST_RESULT
    from concourse.bass_utils import run_bass_kernel_spmd

    cfg = CFG
    in_maps = shard_inputs(fine_feats, coarse_ids, cfg)
    if "full" not in _nc_cache:
        _nc_cache["full"] = build_nc(cfg)
    nc = _nc_cache["full"]
    trace = bool(int(os.environ.get("KERNEL_TRACE", "0")))
    if trace:
        _install_axon_hooks_shim()
    res = run_bass_kernel_spmd(nc, in_maps, core_ids=list(range(NCORES)), trace=trace)
    LAST_RESULT = res
    return assemble_output(res.results, int(num_coarse), cfg)
